# revision 1
# baseline (speedup 1.0000x reference)
"""Trainium2 Bass kernel for nn_Decouple (per-pixel dynamic 3x3 kernel with
dilation 2, then 3x3 conv + bias + LeakyReLU 0.2).

Sharding: pure data parallel over 8 cores; core c handles image n = c//2,
output rows [96*s, 96*s+96) with s = c%2. Inside each core the 96 rows are
split into two 48-row halves stacked on the 128 SBUF partitions
(partitions 0-63 = half A channels, 64-127 = half B channels).

y is pre-arranged on the host into per-(tile, tap-group, half) blocks that
are contiguous per channel, so each DMA moves ~28-32KB contiguous runs per
partition (SDMA per-descriptor overhead amortized).

Pipeline per 12-row tile:
  DMA : 6 y-block loads (3 tap groups x 2 halves)
  DVE : 9 per-tap products, in place over the y slot  (fp32 -> fp32r)
  PE  : 9 identity matmuls accumulate taps in PSUM    (fp32r, K=128)
  ACT : PSUM -> padded SBUF out1 tile (+halo rows from prev tile)
  PE  : 3x3 conv = 9 block-diagonal matmuls into PSUM (fp32r, M=128)
  ACT : Prelu(conv + bias, alpha=0.2) -> SBUF, DMA out
"""
import sys

if "/opt/trn_rl_repo" not in sys.path:
    sys.path.append("/opt/trn_rl_repo")

import json

import numpy as np

import concourse.bass as bass
import concourse.tile as tile
from concourse import mybir
from concourse.bass_utils import run_bass_kernel_spmd

F32 = mybir.dt.float32
F32R = mybir.dt.float32r

N, C, H, W = 4, 64, 192, 192
DIL = 2
N_CORES = 8
HS = H // 2          # rows per core (96)
HH = HS // 2         # rows per half (48)
OUT2 = [4, 12, 12, 12, 8]        # out rows per tile (sum = 48)
T = len(OUT2)
A = [sum(OUT2[:i]) for i in range(T + 1)]  # tile start rows
RMAX = max(OUT2) + 2


def _tile_rows(t):
    # out1 rows computed in tile t
    return OUT2[0] + 2 if t == 0 else OUT2[t]


def _y_block_offsets():
    """(t, g) -> float offset into the flat y-prep array (128-partition blocks)."""
    offs = {}
    off = 0
    for t in range(T):
        rows = _tile_rows(t)
        for g in range(3):
            offs[(t, g)] = off
            off += 2 * C * 3 * rows * W
    return offs, off


_Y_OFFS, _Y_TOTAL = _y_block_offsets()


def _legalize_waits(nc):
    """This container's walrus accepts at most ONE sync wait per instruction.
    Split any instruction with k>1 waits into k-1 single-wait NoOps inserted
    immediately before it on the same engine."""
    raw = json.loads(type(nc).to_json_bytes(nc))
    counter = [0]
    for func in raw.get("functions", []):
        for blk in func.get("blocks", []):
            new_insts = []
            for inst in blk.get("instructions", []):
                si = inst.get("sync_info")
                waits = (si or {}).get("on_wait") or []
                if len(waits) > 1:
                    for w in waits[:-1]:
                        counter[0] += 1
                        new_insts.append(
                            {
                                "engine": inst["engine"],
                                "ins": [],
                                "name": f"wsplit_{counter[0]}",
                                "opcode": "NoOp",
                                "outs": [],
                                "sync_info": {"on_update": [], "on_wait": [w]},
                            }
                        )
                    si["on_wait"] = [waits[-1]]
                new_insts.append(inst)
            blk["instructions"] = new_insts
    fixed = json.dumps(raw).encode()
    nc.to_json_bytes = lambda: fixed


def build_nc():
    nc = bass.Bass()
    xin = nc.declare_dram_parameter("xin", [2 * C, HH + 6, W + 4], F32, isOutput=False)
    yp = nc.declare_dram_parameter("yp", [_Y_TOTAL], F32R, isOutput=False)
    w9 = nc.declare_dram_parameter("w9", [9, 128, 128], F32R, isOutput=False)
    ident = nc.declare_dram_parameter("ident", [128, 128], F32R, isOutput=False)
    bias = nc.declare_dram_parameter("bias", [128, 1], F32, isOutput=False)
    zeros = nc.declare_dram_parameter("zeros", [128, RMAX], F32R, isOutput=False)
    out = nc.declare_dram_parameter("out", [2 * C, HH, W], F32, isOutput=True)

    XROWS = HH + 6  # x rows per half (54)

    with tile.TileContext(nc) as tc:
        with (
            tc.tile_pool(name="consts", bufs=1) as consts,
            tc.tile_pool(name="ypool", bufs=3) as ypool,
            tc.tile_pool(name="out1p", bufs=2) as out1p,
            tc.tile_pool(name="out2p", bufs=2) as out2p,
            tc.tile_pool(name="ps1", bufs=1, space="PSUM") as ps1,
            tc.tile_pool(name="ps2", bufs=3, space="PSUM") as ps2,
        ):
            w_sb = consts.tile([128, 9, 128], F32R)
            nc.sync.dma_start(w_sb[:], w9.rearrange("t p m -> p t m"))
            id_sb = consts.tile([128, 128], F32R)
            nc.sync.dma_start(id_sb[:], ident[:])
            b_sb = consts.tile([128, 1], F32)
            nc.sync.dma_start(b_sb[:], bias[:])
            zc = consts.tile([128, RMAX], F32R)
            nc.sync.dma_start(zc[:], zeros[:])

            xp = consts.tile([128, XROWS, W + 4], F32)
            # split: first piece covers tile 0's taps (rows 0..18), rest after
            nc.gpsimd.dma_start(xp[:, 0:20, :], xin[:, 0:20, :])
            nc.gpsimd.dma_start(xp[:, 20:XROWS, :], xin[:, 20:XROWS, :])

            prev_o1 = None
            prev_rows2 = 0
            for t in range(T):
                rows = _tile_rows(t)
                r2 = OUT2[t]
                nfree = rows * W

                # ---- y loads: 3 tap-groups x 2 halves, contiguous blocks ----
                ysl = []
                for g in range(3):
                    ys = ypool.tile([128, 3, rows, W], F32R, tag="y")
                    off = _Y_OFFS[(t, g)]
                    blk = 2 * C * 3 * rows * W
                    src = yp[off : off + blk].rearrange("(c f) -> c f", c=2 * C)
                    dst = ys.rearrange("p a b c -> p (a b c)")
                    nc.gpsimd.dma_start(dst, src)
                    ysl.append(ys)

                # ---- products (in place) + 9-tap reduction into PSUM ----
                o1 = out1p.tile([128, r2 + 2, W + 2], F32R, tag="out1")
                r0 = 0 if t == 0 else 2
                p1 = ps1.tile([128, rows * W], F32, tag="p1")
                for k in range(9):
                    g, kk = divmod(k, 3)
                    di, dj = divmod(k, 3)
                    x0 = (2 * di) if t == 0 else (A[t] + 2 * di + 2)
                    pv = ysl[g][:, kk, :, :]
                    nc.vector.tensor_tensor(
                        pv,
                        xp[:, x0 : x0 + rows, 2 * dj : 2 * dj + W],
                        pv.bitcast(F32),
                        op=mybir.AluOpType.mult,
                    )
                    prf = pv.rearrange("p r w -> p (r w)")
                    for c0 in range(0, nfree, 512):
                        cn = min(512, nfree - c0)
                        nc.tensor.matmul(
                            p1[:, c0 : c0 + cn],
                            id_sb[:],
                            prf[:, c0 : c0 + cn],
                            start=(k == 0),
                            stop=(k == 8),
                        )
                p1v = p1.rearrange("p (r w) -> p r w", w=W)
                nc.scalar.copy(o1[:, r0 : r0 + rows, 1 : W + 1], p1v[:])
                if t > 0:
                    nc.scalar.copy(
                        o1[:, 0:2, :],
                        prev_o1[:, prev_rows2 : prev_rows2 + 2, :].bitcast(F32),
                    )
                # zero pad columns
                zv = zc.rearrange("p (r o) -> p r o", o=1)
                nc.scalar.copy(o1[:, :, 0:1], zv[:, 0 : r2 + 2, :].bitcast(F32))
                nc.scalar.copy(
                    o1[:, :, W + 1 : W + 2], zv[:, 0 : r2 + 2, :].bitcast(F32)
                )
                prev_o1 = o1
                prev_rows2 = r2

                # ---- conv: 2-row chunks, 9 block-diagonal matmuls each ----
                o2 = out2p.tile([128, r2, W], F32, tag="out2")
                for j in range(r2 // 2):
                    p2 = ps2.tile([128, 2, W], F32, tag="p2")
                    for tp in range(9):
                        ki, kj = divmod(tp, 3)
                        nc.tensor.matmul(
                            p2[:],
                            w_sb[:, tp, :],
                            o1[:, 2 * j + ki : 2 * j + ki + 2, kj : kj + W],
                            start=(tp == 0),
                            stop=(tp == 8),
                        )
                    nc.scalar.activation(
                        o2[:, 2 * j : 2 * j + 2, :],
                        p2[:],
                        mybir.ActivationFunctionType.Prelu,
                        bias=b_sb[:, 0:1],
                        scale=1.0,
                        alpha=0.2,
                    )

                # ---- store: one 128-partition DMA per tile ----
                nc.sync.dma_start(out[:, A[t] : A[t] + r2, :], o2[:])
    _legalize_waits(nc)
    return nc


_NC_CACHE = None


def _get_nc():
    global _NC_CACHE
    if _NC_CACHE is None:
        _NC_CACHE = build_nc()
    return _NC_CACHE


def _prep_core_inputs(x, y, n, s):
    h0 = s * HS
    xpad = np.zeros((C, HS + 6, W + 4), dtype=np.float32)
    a, b = max(0, h0 - 3), min(H, h0 + HS + 3)
    xpad[:, a - (h0 - 3) : b - (h0 - 3), 2 : W + 2] = x[n][:, a:b, :]
    # stacked halves: [2, C, 54, W+4] -> [128, 54, W+4]
    XR = HH + 6
    xin = np.concatenate(
        [xpad[:, 0:XR, :], xpad[:, HH : HH + XR, :]], axis=0
    ).reshape(2 * C, XR, W + 4)

    # padded y rows [h0-1, h0+97), as [C, 9, 98, W]
    yin = np.zeros((C, 9, HS + 2, W), dtype=np.float32)
    a, b = max(0, h0 - 1), min(H, h0 + HS + 1)
    yin[:, :, a - (h0 - 1) : b - (h0 - 1), :] = y[n].reshape(C, 9, H, W)[
        :, :, a:b, :
    ]

    ypf = np.empty(_Y_TOTAL, dtype=np.float32)
    for t in range(T):
        rows = _tile_rows(t)
        r0 = 0 if t == 0 else A[t] + 2
        for g in range(3):
            off = _Y_OFFS[(t, g)]
            blk = C * 3 * rows * W
            for half in range(2):
                rr = r0 + HH * half
                ypf[off + half * blk : off + (half + 1) * blk] = yin[
                    :, 3 * g : 3 * g + 3, rr : rr + rows, :
                ].reshape(-1)
    return xin, ypf


def _prep_weights(fuse_w, fuse_b):
    w9 = np.zeros((9, 128, 128), dtype=np.float32)
    for tp in range(9):
        ki, kj = divmod(tp, 3)
        wt = fuse_w[:, :, ki, kj].T  # [i, o]
        w9[tp, 0:64, 0:64] = wt
        w9[tp, 64:128, 64:128] = wt
    ident = np.eye(128, dtype=np.float32)
    bias = np.concatenate([fuse_b, fuse_b]).reshape(128, 1).astype(np.float32)
    zeros = np.zeros((128, RMAX), dtype=np.float32)
    return w9, ident, bias, zeros


def kernel(x, y, fuse_w, fuse_b):
    x = np.asarray(x, dtype=np.float32)
    y = np.asarray(y, dtype=np.float32)
    fuse_w = np.asarray(fuse_w, dtype=np.float32)
    fuse_b = np.asarray(fuse_b, dtype=np.float32)

    w9, ident, bias, zeros = _prep_weights(fuse_w, fuse_b)

    in_maps = []
    for c in range(N_CORES):
        n, s = divmod(c, 2)
        xin, ypf = _prep_core_inputs(x, y, n, s)
        in_maps.append(
            {
                "xin": xin,
                "yp": ypf,
                "w9": w9,
                "ident": ident,
                "bias": bias,
                "zeros": zeros,
            }
        )

    nc = _get_nc()
    res = run_bass_kernel_spmd(nc, in_maps, list(range(N_CORES)))

    full = np.empty((N, C, H, W), dtype=np.float32)
    for c in range(N_CORES):
        n, s = divmod(c, 2)
        o4 = res.results[c]["out"].reshape(2, C, HH, W)
        for half in range(2):
            r = s * HS + half * HH
            full[n, :, r : r + HH, :] = o4[half]
    return full

